# revision 16
# baseline (speedup 1.0000x reference)
"""Depth-to-space (CRD order) kernel for Trainium2, 8 NeuronCores.

in:  (32, 9, 512, 512) f32, channel c = r*3+s encodes (row_off, col_off)
out: (32, 1, 1536, 1536) f32 with out[b,0,3i+r,3j+s] = in[b,3r+s,i,j]

Sharding: data-parallel over batch, 4 batches per core, no communication.
Per core per (batch, 128-row chunk, row-offset r):
  - DMA-in  x[b, 3r:3r+3, i0:i0+128, :] -> SBUF [128, 3*512]    (768 KB,
    SP HWDGE ring; partition = image row, 2KB runs)
  - one strided-AP DVE copy interleaving the 3 channels into contiguous
    output rows: out[p, 3j+s] = in[p, s*512+j]
  - DMA-out [128, 1536] -> y rows 3*i0+r .. stride 3             (768 KB,
    ACT HWDGE ring; 6KB contiguous runs)
Loads and stores ride separate HWDGE rings so neither blocks the other
(FIFO per ring); measured ~197 us/core = ~94% of the 435 GB/s per-core
SBUF-port ceiling incl. ~11 us fixed NEFF preamble.
"""

import sys

import numpy as np

_B, _C, _H, _W = 32, 9, 512, 512
_K = 3
_NCORES = 8
_BLOC = _B // _NCORES  # 4

_PROG = None


def _ensure_path():
    try:
        import concourse.bass  # noqa: F401
    except ImportError:
        sys.path.insert(0, "/opt/trn_rl_repo")


def _build():
    import concourse.bacc as bacc
    import concourse.mybir as mybir
    from concourse import tile

    f32 = mybir.dt.float32
    nc = bacc.Bacc(None)
    x = nc.declare_dram_parameter("x", [_BLOC, _C, _H, _W], f32, isOutput=False)
    y = nc.declare_dram_parameter("y", [_BLOC, _K * _H, _K * _W], f32, isOutput=True)

    P = 128
    KW = _K * _W  # 1536

    with tile.TileContext(nc) as tc:
        with (
            tc.tile_pool(name="tin", bufs=6) as pin,
            tc.tile_pool(name="tout", bufs=3) as pout,
        ):
            su = 0
            n_units = _BLOC * (_H // P)
            for b in range(_BLOC):
                for i0 in range(0, _H, P):
                    unit = su // _K
                    # all 3 copies land on DVE, so the merged store needs only
                    # a single sem tick (the last copy) — 18KB contiguous runs
                    tout = pout.tile([P, _K * KW], f32)
                    for r in range(_K):
                        # dedicated HWDGE rings: SP carries loads, ACT stores;
                        # mixing them on one ring lets a not-yet-ready store
                        # block ready loads behind it (FIFO per ring). The
                        # edges are safe exceptions: first loads ride the
                        # still-idle store ring (no stores queued yet).
                        ld_eng = nc.scalar if su < 2 else nc.sync
                        su += 1
                        # copy r consumes exactly channels 3r..3r+2
                        tin = pin.tile([P, KW], f32)
                        ld_eng.dma_start(
                            out=tin[:].rearrange("p (s j) -> p s j", s=_K),
                            in_=x[b, _K * r : _K * (r + 1), i0 : i0 + P, :].rearrange(
                                "s p j -> p s j"
                            ),
                        )
                        # out[p, 3j+s] = in[p, s*512+j]
                        nc.vector.tensor_copy(
                            out=tout[:, r * KW : (r + 1) * KW].rearrange(
                                "p (j s) -> p j s", s=_K
                            ),
                            in_=tin[:].rearrange("p (s j) -> p j s", s=_K),
                        )
                    # rows 3*i0 .. 3*i0+384: one fully contiguous 2.25MB store;
                    # the last store rides the by-then-drained load ring
                    st_eng = nc.sync if unit == n_units - 1 else nc.scalar
                    st_eng.dma_start(
                        out=y[b, _K * i0 : _K * (i0 + P), :].rearrange(
                            "(p r) w -> p (r w)", r=_K
                        ),
                        in_=tout[:],
                    )
    return nc


def _run(x_full, trace=False, **spmd_kwargs):
    """x_full: (32, 9, 512, 512) f32 ndarray. Returns (out, BassKernelResults)."""
    global _PROG
    _ensure_path()
    from concourse.bass_utils import run_bass_kernel_spmd

    if _PROG is None:
        _PROG = _build()
        if not _PROG.is_finalized():
            _PROG.finalize()
    in_maps = [
        {"x": np.ascontiguousarray(x_full[i * _BLOC : (i + 1) * _BLOC])}
        for i in range(_NCORES)
    ]
    res = run_bass_kernel_spmd(
        _PROG, in_maps, core_ids=list(range(_NCORES)), trace=trace, **spmd_kwargs
    )
    out = np.concatenate([np.asarray(r["y"]) for r in res.results], axis=0)
    return out.reshape(_B, 1, _K * _H, _K * _W), res


def kernel(**inputs):
    x = np.ascontiguousarray(np.asarray(inputs["inputs"], dtype=np.float32))
    k = int(np.asarray(inputs.get("kernel_size", _K)))
    assert k == _K, f"kernel hardcodes kernel_size=3, got {k}"
    assert x.shape == (_B, _C, _H, _W), x.shape
    out, _ = _run(x)
    return out



# revision 18
# speedup vs baseline: 1.1823x; 1.1823x over previous
"""Depth-to-space (CRD order) kernel for Trainium2, 8 NeuronCores.

in:  (32, 9, 512, 512) f32, channel c = r*3+s encodes (row_off, col_off)
out: (32, 1, 1536, 1536) f32 with out[b,0,3i+r,3j+s] = in[b,3r+s,i,j]

Sharding: data-parallel over batch, 4 batches per core, no communication.
Per core per (batch, 128-row chunk, row-offset r):
  - DMA-in  x[b, 3r:3r+3, i0:i0+128, :] -> SBUF [128, 3*512]    (768 KB,
    SP HWDGE ring; partition = image row, 2KB runs)
  - one strided-AP DVE copy interleaving the 3 channels into contiguous
    output rows: out[p, 3j+s] = in[p, s*512+j]
  - DMA-out [128, 1536] -> y rows 3*i0+r .. stride 3             (768 KB,
    ACT HWDGE ring; 6KB contiguous runs)
Loads and stores ride separate HWDGE rings so neither blocks the other
(FIFO per ring); measured ~197 us/core = ~94% of the 435 GB/s per-core
SBUF-port ceiling incl. ~11 us fixed NEFF preamble.
"""

import sys

import numpy as np

_B, _C, _H, _W = 32, 9, 512, 512
_K = 3
_NCORES = 8
_BLOC = _B // _NCORES  # 4

_PROG = None


def _ensure_path():
    try:
        import concourse.bass  # noqa: F401
    except ImportError:
        sys.path.insert(0, "/opt/trn_rl_repo")


def _build():
    import concourse.bacc as bacc
    import concourse.mybir as mybir
    from concourse import tile

    f32 = mybir.dt.float32
    nc = bacc.Bacc(None)
    x = nc.declare_dram_parameter("x", [_BLOC, _C, _H, _W], f32, isOutput=False)
    y = nc.declare_dram_parameter("y", [_BLOC, _K * _H, _K * _W], f32, isOutput=True)

    P = 128
    KW = _K * _W  # 1536

    with tile.TileContext(nc) as tc:
        with (
            tc.tile_pool(name="tin", bufs=6) as pin,
            tc.tile_pool(name="tout", bufs=6) as pout,
        ):
            su = 0
            for b in range(_BLOC):
                for i0 in range(0, _H, P):
                    # output rows 3*i0 .. 3*i0+384, grouped by row offset r
                    dst = y[b, _K * i0 : _K * (i0 + P), :].rearrange(
                        "(p r) w -> r p w", r=_K
                    )
                    for r in range(_K):
                        # dedicated HWDGE rings: SP carries loads, ACT stores;
                        # mixing them on one ring lets a not-yet-ready store
                        # block ready loads behind it (FIFO per ring). The
                        # edges are safe exceptions: first loads ride the
                        # still-idle store ring, last stores the drained load
                        # ring (no younger work queues behind them there).
                        # ramp: alternate the first 6 loads across both rings
                        # (store ring is still empty, so no store can block
                        # them); drain: last stores ride the drained load ring
                        ld_eng = (nc.sync, nc.scalar)[su % 2] if su < 6 else nc.sync
                        st_eng = nc.sync if su >= 46 else nc.scalar
                        su += 1
                        # copy r consumes exactly channels 3r..3r+2
                        tin = pin.tile([P, KW], f32)
                        ld_eng.dma_start(
                            out=tin[:].rearrange("p (s j) -> p s j", s=_K),
                            in_=x[b, _K * r : _K * (r + 1), i0 : i0 + P, :].rearrange(
                                "s p j -> p s j"
                            ),
                        )
                        # out[p, 3j+s] = in[p, s*512+j]
                        tout = pout.tile([P, KW], f32)
                        nc.vector.tensor_copy(
                            out=tout[:].rearrange("p (j s) -> p j s", s=_K),
                            in_=tin[:].rearrange("p (s j) -> p j s", s=_K),
                        )
                        st_eng.dma_start(out=dst[r], in_=tout[:])
    return nc


def _run(x_full, trace=False, **spmd_kwargs):
    """x_full: (32, 9, 512, 512) f32 ndarray. Returns (out, BassKernelResults)."""
    global _PROG
    _ensure_path()
    from concourse.bass_utils import run_bass_kernel_spmd

    if _PROG is None:
        _PROG = _build()
        if not _PROG.is_finalized():
            _PROG.finalize()
    in_maps = [
        {"x": np.ascontiguousarray(x_full[i * _BLOC : (i + 1) * _BLOC])}
        for i in range(_NCORES)
    ]
    res = run_bass_kernel_spmd(
        _PROG, in_maps, core_ids=list(range(_NCORES)), trace=trace, **spmd_kwargs
    )
    out = np.concatenate([np.asarray(r["y"]) for r in res.results], axis=0)
    return out.reshape(_B, 1, _K * _H, _K * _W), res


def kernel(**inputs):
    x = np.ascontiguousarray(np.asarray(inputs["inputs"], dtype=np.float32))
    k = int(np.asarray(inputs.get("kernel_size", _K)))
    assert k == _K, f"kernel hardcodes kernel_size=3, got {k}"
    assert x.shape == (_B, _C, _H, _W), x.shape
    out, _ = _run(x)
    return out

